# revision 4
# baseline (speedup 1.0000x reference)
"""Trainium2 Bass kernel for nn_Attention_40475771798025 (v2).

Full attention layer: QKV projection + RoPE + GQA causal attention + output
projection. B=2, S=2048, D=4096, H=32 q-heads, KV=8 kv-heads, HD=128.

Sharding: head-parallel tensor parallelism across 8 cores. Core g owns kv-head
g (its 4 q-heads, 1 k-head, 1 v-head) for both batches; the output projection
produces per-core partial sums of the full [T, D] output, summed on the host.

v2 design (vs v1): bf16 datapath everywhere except PSUM accumulation (f32)
and softmax-denominator accumulation (f32). Weights (qkv + wo) are resident
in SBUF in bf16, x streams in bf16. QKV projection accumulates the full
D=4096 contraction in PSUM (6 m-chains per 512-token tile across 8 passes),
eliminating the SBUF-accumulation DVE traffic of v1. RoPE + V-transpose run
per token-tile right after evacuation. Attention is causally trimmed at
128-column granularity; softmax denominators accumulate on DVE (e-tile adds)
with a single ones-matmul per (head, q-tile), freeing ~1/3 of phase-B PE
cycles. The output projection for each 512-token q-tile is emitted right
after its attention completes, filling PE bubbles and spreading the output
DMA. PSUM: tags chain(6)/aux(1)/o(1) = 16KB/partition exactly.
"""
import sys
sys.path.insert(0, "/opt/trn_rl_repo")
import numpy as np
import ml_dtypes

B, S, D = 2, 2048, 4096
H, KV, HD = 32, 8, 128
REP = H // KV            # 4 q-heads per core
T = B * S                # 4096 flattened tokens
NCORES = 8
P = 128
QTW = 512                # q/token tile width
NT = S // QTW            # 4 token tiles per batch
CPP = 4                  # D-chunks (of 128) per phase-A pass
NPASS = (D // P) // CPP  # 8 passes
MQKV = REP + 2           # q0..q3, k, v
KSLOT, VSLOT = REP, REP + 1
SCALE = 1.0 / float(np.sqrt(HD))
PIPE = 4                 # phase-B EV pipeline depth

_nc = None


def _build_nc(reps=1):
    import concourse.bacc as bacc
    import concourse.mybir as mybir
    import concourse.tile as tile
    from contextlib import ExitStack

    F32 = mybir.dt.float32
    F32R = mybir.dt.float32r
    BF16 = mybir.dt.bfloat16
    EXP = mybir.ActivationFunctionType.Exp

    nc = bacc.Bacc("TRN2")
    xT_d = nc.dram_tensor("xT", (D, T), BF16, kind="ExternalInput")
    wqkvT_d = nc.dram_tensor("wqkvT", (D, MQKV * P), BF16,
                             kind="ExternalInput")
    woT_d = nc.dram_tensor("woT", (REP * P, D), BF16, kind="ExternalInput")
    cdup_d = nc.dram_tensor("cdup", (P, T), F32, kind="ExternalInput")
    sdup_d = nc.dram_tensor("sdup", (P, T), F32, kind="ExternalInput")
    ptb_d = nc.dram_tensor("ptb", (P, P), BF16, kind="ExternalInput")
    ones_d = nc.dram_tensor("ones", (P, P), F32, kind="ExternalInput")
    identb_d = nc.dram_tensor("identb", (P, P), BF16, kind="ExternalInput")
    maskb_d = nc.dram_tensor("maskb", (P, P), F32, kind="ExternalInput")
    out_d = nc.dram_tensor("out", (T, D), BF16, kind="ExternalOutput")

    with tile.TileContext(nc) as tc, ExitStack() as top:
        persist = top.enter_context(tc.tile_pool(name="persist", bufs=1))
        csp = top.enter_context(tc.tile_pool(name="cs", bufs=1))
        xp = top.enter_context(tc.tile_pool(name="x", bufs=4))
        tmpp = top.enter_context(tc.tile_pool(name="tmp", bufs=2))
        ep = top.enter_context(tc.tile_pool(name="e", bufs=PIPE + 6))
        esp = top.enter_context(tc.tile_pool(name="es", bufs=3))
        rp = top.enter_context(tc.tile_pool(name="rec", bufs=2))
        atp = top.enter_context(tc.tile_pool(name="att", bufs=3))
        obp = top.enter_context(tc.tile_pool(name="ob", bufs=4))
        psp = top.enter_context(
            tc.tile_pool(name="ps", bufs=1, space="PSUM"))

        ptb_s = persist.tile([P, P], BF16)
        ones_s = persist.tile([P, P], F32R)
        identb_s = persist.tile([P, P], BF16)
        maskb_s = persist.tile([P, P], F32)

        # resident weights: qkv in D-chunk granularity so the first matmuls
        # start after one chunk's DMA, not the full 6.3MB; chunk 0 before the
        # small persist loads so the very first matmul isn't DMA-gated
        wq_s = persist.tile([P, D // P, MQKV * P], BF16)
        wsrc = wqkvT_d[:].rearrange("(c p) m -> p c m", p=P)
        for c in range(CPP):
            nc.scalar.dma_start(wq_s[:, c:c + 1, :], wsrc[:, c:c + 1, :])
        nc.scalar.dma_start(ptb_s[:], ptb_d[:])
        nc.scalar.dma_start(ones_s[:], ones_d[:].bitcast(F32R))
        nc.scalar.dma_start(identb_s[:], identb_d[:])
        nc.scalar.dma_start(maskb_s[:], maskb_d[:])
        for pp in range(1, NPASS):
            cs = slice(pp * CPP, (pp + 1) * CPP)
            nc.scalar.dma_start(wq_s[:, cs, :], wsrc[:, cs, :])
        wo_s = persist.tile([P, REP, D], BF16)

        acc = persist.tile([P, MQKV, S], BF16)
        v_nat = persist.tile([P, S // P, HD], BF16)

        for _rep in range(reps):
          c_backlog = []
          finish_q = []
          ktstep = 0
          for b in range(B):
            bsl = slice(b * S, (b + 1) * S)

            cdup_b = csp.tile([P, NT, QTW], F32, tag="c")
            sdup_b = csp.tile([P, NT, QTW], F32, tag="s")

            # ---- phase A: qkv projection + rope + v transpose ----
            for tq in range(NT):
                tqsl = slice(tq * QTW, (tq + 1) * QTW)
                if _rep == 0 and b == 0 and tq > 0:
                    # wo is resident and first needed ~180us in; trickle it
                    # in 0.5MB chunks, skipping tq0 where x-stream prefetch
                    # still competes for the DMA engines
                    nu = (5, 5, 6)[tq - 1]
                    u0 = (0, 5, 10)[tq - 1]
                    for uu in range(u0, u0 + nu):
                        hh, q4 = uu // 4, uu % 4
                        nc.scalar.dma_start(
                            wo_s[:, hh, q4 * 1024:(q4 + 1) * 1024],
                            woT_d[hh * P:(hh + 1) * P,
                                  q4 * 1024:(q4 + 1) * 1024])
                ch = [psp.tile([P, QTW], F32, tag="chain", bufs=6,
                               name=f"ch{m}")
                      for m in range(MQKV)]
                for pp in range(NPASS):
                    xq = xp.tile([P, CPP, QTW], BF16, tag="x")
                    xsrc = (xT_d[pp * CPP * P:(pp + 1) * CPP * P,
                                 b * S + tq * QTW:b * S + (tq + 1) * QTW]
                            .rearrange("(c p) t -> p c t", p=P))
                    if _rep == 0 and b == 0 and tq == 0 and pp == 0:
                        # kernel start: per-chunk sub-DMAs so the first
                        # matmul is gated on 0.13MB, not the full tile
                        for c in range(CPP):
                            nc.sync.dma_start(xq[:, c:c + 1, :],
                                              xsrc[:, c:c + 1, :])
                    else:
                        nc.sync.dma_start(xq[:], xsrc[:])
                    for m in range(MQKV):
                        for c in range(CPP):
                            nc.tensor.matmul(
                                ch[m][:],
                                lhsT=wq_s[:, pp * CPP + c,
                                          m * P:(m + 1) * P],
                                rhs=xq[:, c, :],
                                start=(pp == 0 and c == 0),
                                stop=(pp == NPASS - 1 and c == CPP - 1))
                if tq == 0:
                    # rope tables: first needed right below; emitted after
                    # the chain DMAs so they don't compete with the
                    # x-stream warmup at the start of the batch
                    nc.scalar.dma_start(
                        cdup_b[:],
                        cdup_d[:, bsl].rearrange("p (n q) -> p n q", q=QTW))
                    nc.scalar.dma_start(
                        sdup_b[:],
                        sdup_d[:, bsl].rearrange("p (n q) -> p n q", q=QTW))
                for i, m in enumerate((0, KSLOT, 1, 2, 3, VSLOT)):
                    if i % 2 == 0:
                        nc.scalar.copy(acc[:, m, tqsl], ch[m][:])
                    else:
                        nc.vector.tensor_copy(acc[:, m, tqsl], ch[m][:])
                # rope in place on k and q0..3 for this token tile
                for m in (0, KSLOT, 1, 2, 3):
                    accsl = acc[:, m, tqsl]
                    # own rotation slot: sharing "aux" with the softmax
                    # denominators serializes rope against phase B
                    rot = psp.tile([P, QTW], F32, tag="chain", bufs=6,
                                   name="rot")
                    nc.tensor.matmul(rot[:], lhsT=ptb_s[:], rhs=accsl,
                                     start=True, stop=True)
                    t1 = tmpp.tile([P, QTW], F32, tag="t1")
                    t2 = tmpp.tile([P, QTW], F32, tag="t2")
                    nc.gpsimd.tensor_mul(t1[:], accsl, cdup_b[:, tq, :])
                    nc.vector.tensor_mul(t2[:], rot[:], sdup_b[:, tq, :])
                    nc.vector.tensor_add(accsl, t1[:], t2[:])
                # v -> natural [token, hd] layout
                for c4 in range(NT):
                    vt_f = psp.tile([P, QTW], F32, tag="chain", bufs=6,
                                    name="vt")
                    vt = vt_f.bitcast(BF16)[:, :P]
                    nc.tensor.transpose(
                        vt, acc[:, VSLOT, tq * QTW + c4 * P:
                                tq * QTW + (c4 + 1) * P], identb_s[:])
                    if c4 % 2 == 0:
                        nc.scalar.copy(v_nat[:, tq * NT + c4, :], vt)
                    else:
                        nc.vector.tensor_copy(v_nat[:, tq * NT + c4, :], vt)

            # ---- phase B+C interleaved per q-tile ----
            # deepest q-tile first: its 16-ktile pipeline fills while the
            # ACT/DVE queues drain phase A's tail; the shallow qt=0 runs last.
            # Output-projection work items for completed q-tiles are drained
            # one at a time between attention k-steps so the PE fills the
            # exp-latency bubbles; leftovers flush after the last q-tile.

            def emit_c_item(item):
                att_t, row0, nq, tt4 = item
                ncol = D // 4
                ob = obp.tile([P, 2 * QTW], BF16, tag="ob", name="ob")
                for half in range(2):
                    ps = psp.tile([P, QTW], F32, tag="chain", bufs=6,
                                  name="ps_c")
                    for hh in range(REP):
                        nc.tensor.matmul(
                            ps[:],
                            lhsT=att_t[:, hh, tt4 * P:(tt4 + 1) * P],
                            rhs=wo_s[:, hh,
                                     nq * ncol + half * QTW:
                                     nq * ncol + (half + 1) * QTW],
                            start=(hh == 0), stop=(hh == REP - 1))
                    # evacuate on DVE: ACT is exp-saturated during phase B
                    nc.vector.tensor_copy(
                        ob[:, half * QTW:(half + 1) * QTW], ps[:])
                    nc.sync.dma_start(
                        out_d[row0:row0 + P,
                              nq * ncol + half * QTW:
                              nq * ncol + (half + 1) * QTW],
                        ob[:, half * QTW:(half + 1) * QTW])

            for qt in (3, 2, 1, 0):
                nkt = (qt + 1) * (QTW // P)
                # pace the C-item drain to spread the current backlog over
                # this whole q-tile instead of running dry partway through
                stride = max(1, (REP * nkt) // max(1, len(c_backlog)))
                qstep = 0
                att = atp.tile([P, REP, QTW], BF16, tag="att")
                for h in range(REP):
                    esum_a = esp.tile([P, QTW], F32, tag="esa", bufs=2)
                    esum_b = (esp.tile([P, QTW], F32, tag="esb", bufs=2,
                                       name="esum_b")
                              if nkt > 4 else None)
                    ps_o = psp.tile([P, QTW], F32, tag="o", bufs=1)
                    pend = []

                    def flush(upto, ps_o=ps_o, nkt=nkt, pend=pend):
                        while len(pend) > upto:
                            pe, pkt, pw0 = pend.pop(0)
                            nc.tensor.matmul(
                                ps_o[:, pw0:], lhsT=v_nat[:, pkt, :],
                                rhs=pe[:, pw0:],
                                start=(pkt == 0), stop=(pkt == nkt - 1),
                                skip_group_check=True)

                    for kt in range(nkt):
                        j = kt - qt * (QTW // P)
                        w0 = P * j if j >= 0 else 0
                        ps_s = psp.tile([P, QTW], F32, tag="chain", bufs=6,
                                        name="ps_s")
                        nc.tensor.matmul(
                            ps_s[:, w0:],
                            lhsT=acc[:, KSLOT, kt * P:(kt + 1) * P],
                            rhs=acc[:, h, qt * QTW + w0:(qt + 1) * QTW],
                            start=True, stop=True)
                        if j >= 0:
                            nc.vector.tensor_add(
                                ps_s[:, w0:w0 + P], ps_s[:, w0:w0 + P],
                                maskb_s[:])
                        e = ep.tile([P, QTW], BF16, tag="e")
                        nc.scalar.activation(e[:, w0:], ps_s[:, w0:], EXP,
                                             scale=SCALE)
                        # two independent denominator chains (DVE even / Pool
                        # odd) so neither engine must match the exp cadence
                        # alone. qt==0 has no full-width odd tile to init the
                        # b-chain from, and is short anyway: single DVE chain.
                        use_b = nkt > 4
                        if kt == 0:
                            nc.vector.tensor_copy(esum_a[:], e[:])
                        elif use_b and kt == 1:
                            nc.gpsimd.tensor_copy(esum_b[:], e[:])
                        elif not use_b or kt % 2 == 0:
                            nc.vector.tensor_add(
                                esum_a[:, w0:], esum_a[:, w0:], e[:, w0:])
                        else:
                            nc.gpsimd.tensor_add(
                                esum_b[:, w0:], esum_b[:, w0:], e[:, w0:])
                        pend.append((e, kt, w0))
                        flush(PIPE)
                        qstep += 1
                        # the deferred finish of the previous head runs a
                        # step into this head's score stream, so its
                        # denominator matmul never stalls the PE; C items
                        # wait until no finish is pending (their att reads
                        # must not be emitted before the normalize writes)
                        if finish_q and kt >= 1:
                            finish_q.pop(0)()
                        elif c_backlog and qstep % stride == 0:
                            emit_c_item(c_backlog.pop(0))
                    flush(0)
                    # merge the two chains and round to f32r in one DVE op
                    # (walrus requires f32r matmul inputs produced as f32r)
                    esr = esp.tile([P, QTW], F32R, tag="ra", bufs=2)
                    if nkt > 4:
                        nc.vector.tensor_add(esr[:], esum_a[:], esum_b[:])
                    else:
                        nc.vector.tensor_copy(esr[:], esum_a[:])

                    def finish(esr=esr, ps_o=ps_o, att=att, h=h):
                        ps_d = psp.tile([P, QTW], F32, tag="aux", bufs=1)
                        nc.tensor.matmul(ps_d[:], lhsT=ones_s[:],
                                         rhs=esr[:],
                                         start=True, stop=True,
                                         skip_group_check=True)
                        rec = rp.tile([P, QTW], F32, tag="rec")
                        nc.vector.reciprocal_approx_fast(rec[:], ps_d[:])
                        nc.vector.tensor_mul(att[:, h, :], ps_o[:], rec[:])

                    finish_q.append(finish)

                # register this q-tile's output-projection work items
                for nq in range(4):
                    for tt4 in range(QTW // P):
                        c_backlog.append(
                            (att, b * S + qt * QTW + tt4 * P, nq, tt4))
            while finish_q:
                finish_q.pop(0)()
            # carry one q-tile's worth of C items into the next batch's
            # deep B(qt3), which otherwise has no fill work; drain fully at
            # the end of the last batch
            keep = 24 if b == 0 else 0
            while len(c_backlog) > keep:
                emit_c_item(c_backlog.pop(0))
    nc.compile()
    return nc


def get_nc():
    global _nc
    if _nc is None:
        _nc = _build_nc()
    return _nc


def make_in_maps(x, freqs_cos, freqs_sin, wq, wk, wv, wo):
    """Host-side prep: transposes, bf16 casts, rope tables, masks, shards."""
    bf = ml_dtypes.bfloat16
    x = np.asarray(x, np.float32)
    fc = np.asarray(freqs_cos, np.float32)
    fs = np.asarray(freqs_sin, np.float32)
    wq = np.asarray(wq, np.float32)
    wk = np.asarray(wk, np.float32)
    wv = np.asarray(wv, np.float32)
    wo = np.asarray(wo, np.float32)

    xT = np.ascontiguousarray(x.reshape(T, D).T.astype(bf))
    cdup = np.ascontiguousarray(np.tile(np.repeat(fc.T, 2, axis=0), (1, B)))
    sdup = np.ascontiguousarray(np.tile(np.repeat(fs.T, 2, axis=0), (1, B)))
    prot = np.zeros((P, P), np.float32)
    for i in range(P // 2):
        prot[2 * i, 2 * i + 1] = -1.0
        prot[2 * i + 1, 2 * i] = 1.0
    ptb = np.ascontiguousarray(prot.T.astype(bf))
    ones = np.ones((P, P), np.float32)
    identb = np.eye(P, dtype=np.float32).astype(bf)
    ki = np.arange(P)[:, None]
    si = np.arange(P)[None, :]
    maskb = np.where(ki > si, np.float32(-1e9), np.float32(0.0))
    maskb = np.ascontiguousarray(maskb.astype(np.float32))

    in_maps = []
    for g in range(NCORES):
        wq_g = wq[g * REP * HD:(g + 1) * REP * HD]
        wk_g = wk[g * HD:(g + 1) * HD]
        wv_g = wv[g * HD:(g + 1) * HD]
        wqkvT = np.ascontiguousarray(
            np.concatenate([wq_g, wk_g, wv_g], 0).T.astype(bf))
        woT = np.ascontiguousarray(
            wo[:, g * REP * HD:(g + 1) * REP * HD].T.astype(bf))
        in_maps.append({
            "xT": xT, "wqkvT": wqkvT, "woT": woT,
            "cdup": cdup, "sdup": sdup, "ptb": ptb, "ones": ones,
            "identb": identb, "maskb": maskb,
        })
    return in_maps


def kernel(x, freqs_cos, freqs_sin, wq, wk, wv, wo):
    from concourse.bass_utils import run_bass_kernel_spmd
    nc = get_nc()
    in_maps = make_in_maps(x, freqs_cos, freqs_sin, wq, wk, wv, wo)
    res = run_bass_kernel_spmd(nc, in_maps, core_ids=list(range(NCORES)))
    out = np.zeros((T, D), np.float64)
    for r in res.results:
        out += r["out"].astype(np.float64)
    return out.astype(np.float32).reshape(B, S, D)

